# revision 1
# baseline (speedup 1.0000x reference)
"""Trainium2 Bass kernel for nn_Attention_Joint_MaxPool.

Math (see reference):
  q = (Wq*scale) @ x                        (B, C, N), heads on rows
  xsr = conv2x2s2(x) ; k = Wk @ BN(xsr)     (B, C, Nk=1024)
  attn = max over keys of q_h . k_h         (B, NH, N)
  s = sum over heads of attn                (B, N)
  out[b,c,n] = (Wproj @ mean_n x)[c] * s[b,n] + bproj[c]

Weight folding done on host:
  g = gamma/sqrt(var+eps); A = Wk * g[None,:]
  k = sum_e (A @ Wsr[:,:,e]) @ x_sub[e] + ck,  ck = A@bsr + Wk@(beta-mean*g)
  pv[b] = Wproj @ mean_n x[b]   (rank-1 output structure)

Key structure:
  All heavy matmuls run in bf16 (tolerance is 2e-2; bf16 lands ~2.4e-3).
  Max over keys via pair cascade: for key pairs (even, odd):
    max(a,b) = a + relu(b-a); a = q.k_even, (b-a) = q.(k_odd-k_even)
  The cascade is built with ZERO extra PE columns using a PSUM
  has_written trick validated on HW: the diff-pair matmuls write a bank
  (setting its has_written bits), ScalarE relus the bank IN PLACE
  (PSUM->PSUM, which leaves has_written set), and the even-pair matmuls
  then accumulate onto it with start=False -- yielding relu(D)+E, the
  pair max, directly. VectorE reduce_maxes the bank. (The DVE has a
  single PSUM read port and this toolchain's walrus miscompiles every
  fused-reduce ISA op -- stock TENSOR_TENSOR_REDUCE and the ant
  custom-DVE path both die with "ISA wrong length" -- so the PSUM
  egress is split ~50/50 between ScalarE relus and VectorE reduces; a
  few groups' relus run on VectorE to keep the engines level.)
  Score matmuls are K=64 and run two heads concurrently in the PE via
  tile_position row pairing; PE rhs streaming is ~1 col/cycle
  aggregate, so PE busy is ~96us and all engines sit near ~95us.

Pipeline: q chunk 0 -> k head-pair 0 (rides the conv-data DMA) -> score
passes m=0..3 over 16 token tiles each, software-pipelined one group
deep (diff+relu of group g issue alongside the accumulate+reduce of
g-1, hiding the relu latency). The remaining q chunks (one
PSUM-accumulation per group slot) and the next k head-pair's matmuls
are interleaved between score groups; the rank-1 output stage is split
into four parts spread across the last pass.

Sharding: 8 cores; core i -> batch i//2, token half i%2 (2048 tokens).
Each core is fully independent (no collectives).
"""

import os
import sys
import types
import numpy as np

# ---------------------------------------------------------------------------
# problem constants (hardcoded; kernel.py must be self-contained)
# ---------------------------------------------------------------------------
B, C, N = 4, 512, 4096
NH, HD = 8, 64
SR = 2
EPS = 1e-5
HW_ = 64                      # H = W = 64
T = N // 2                    # tokens per core
NK = 1024                     # conv output positions (keys)
NKE = NK // 2                 # even keys
MB = C // 128                 # 4 channel blocks
KC = C // 128                 # 4 contraction chunks
NCORES = 8
TT = T // 128                 # 16 token tiles per core
NCH = T // 512                # 4 q chunks per core

_cache = {}


# ---------------------------------------------------------------------------
# workarounds for this container's toolchain
# ---------------------------------------------------------------------------
def _install_fixes():
    import concourse.tile as tile
    import concourse.mybir as mybir
    from concourse.vector_clock import ScopedClock

    if getattr(tile.TileContext, "_drain_patched", False):
        return

    def _patched_drain_and_barrier(self, tick_clock, wait_clock):
        nc = self.nc
        probe = nc.sync.nop(nofuse=True, hint="drain_wait_carrier")
        wait_clock.add_sem_waits(
            probe.ins, ScopedClock({None: tick_clock.global_clock})
        )
        waits = list(probe.ins.sync_info.on_wait) if probe.ins.sync_info else []
        if len(waits) > 1:
            probe.ins.sync_info = mybir.SyncInfo(on_wait=waits[:1], on_update=[])
            for w in waits[1:]:
                extra = nc.sync.nop(nofuse=True, hint="drain_wait_carrier")
                extra.ins.sync_info = mybir.SyncInfo(on_wait=[w], on_update=[])
        nc.sync.drain()
        nc.all_engine_barrier()
        assert self.sems is not None
        popped = nc._tile_sem_poison_stack.pop()
        assert popped is self._sem_poison
        nc.clear_and_free_semaphores(list(self.sems.allocated().values()))
        nc.all_engine_barrier()

    tile.TileContext._drain_and_barrier = _patched_drain_and_barrier
    tile.TileContext._drain_patched = True


def _split_multi_waits(nc):
    """This walrus build allows only one sync-wait per instruction; hoist
    extra waits onto same-engine nops inserted just before the instruction."""
    import concourse.mybir as mybir

    ctr = 0
    for f in nc.m.functions:
        for bb in f.blocks:
            changed = False
            out = []
            for inst in bb.instructions:
                si = inst.sync_info
                tname = type(inst).__name__
                if (si is not None and si.on_wait and len(si.on_wait) > 1
                        and "Collective" not in tname):
                    waits = list(si.on_wait)
                    for w in waits[:-1]:
                        ctr += 1
                        nop = mybir.InstNoOp(
                            name=f"I-ws-{ctr}",
                            engine=inst.engine,
                            sync_info=mybir.SyncInfo(on_wait=[w], on_update=[]),
                        )
                        nc.register_instruction(nop, overwrite=True)
                        out.append(nop)
                    inst.sync_info = mybir.SyncInfo(
                        on_wait=waits[-1:], on_update=list(si.on_update)
                    )
                    changed = True
                out.append(inst)
            if changed:
                bb.instructions = out


def _install_ntff_hook():
    """Provide antenv.axon_hooks (missing in this image) so trace=True works."""
    try:
        from antenv import axon_hooks  # noqa: F401
        return
    except ImportError:
        pass
    try:
        import antenv
        from trn_agent_boot.trn_boot import _ntff_profile_via_ctypes
    except ImportError:
        return
    mod = types.ModuleType("antenv.axon_hooks")
    _hook = [None]
    mod.set_axon_ntff_profile_hook = lambda h: _hook.__setitem__(0, h)
    mod.get_axon_ntff_profile_hook = lambda: _hook[0]
    sys.modules["antenv.axon_hooks"] = mod
    antenv.axon_hooks = mod
    mod.set_axon_ntff_profile_hook(
        _ntff_profile_via_ctypes("/opt/axon/libaxon_pjrt.so")
    )


# ---------------------------------------------------------------------------
# device program
# ---------------------------------------------------------------------------
def _build_program():
    import concourse.bass as bass
    import concourse.mybir as mybir
    import concourse.tile as tile

    F32 = mybir.dt.float32
    F32R = mybir.dt.float32r
    BF16 = mybir.dt.bfloat16
    AX = mybir.AxisListType
    ACTF = mybir.ActivationFunctionType

    nc = bass.Bass()

    qpack_in = nc.declare_dram_parameter("qpack", [128, 4096], BF16,
                                          isOutput=False)
    convA_in = nc.declare_dram_parameter("convA", [128, 8192], BF16,
                                         isOutput=False)
    convB_in = nc.declare_dram_parameter("convB", [128, 8192], BF16,
                                         isOutput=False)
    wkpack_in = nc.declare_dram_parameter("wkpack", [128, 8192], BF16,
                                          isOutput=False)
    xqrest_in = nc.declare_dram_parameter("xqrest", [128, 6144], BF16,
                                          isOutput=False)
    cpb_in = nc.declare_dram_parameter("cpb", [128, 3 * MB], F32, isOutput=False)
    ones_in = nc.declare_dram_parameter("ones", [1, 128], F32R, isOutput=False)
    out_ext = nc.declare_dram_parameter("out", [C, T], F32, isOutput=True)

    sbounce = nc.dram_tensor("sbounce", [128, TT], F32)

    # groups whose diff-relu runs on VectorE instead of ScalarE (keeps the
    # engines level while ScalarE also runs q copies / the output stage)
    # in-place PSUM relu on the DVE is ~2x the modeled cost (read-modify-
    # write through the single PSUM port): keep every relu on ScalarE.
    VRELU = set()
    # output-stage parts [lo_tt, hi_tt): the DMA-gather + broadcast matmul
    # (start) issue a few slots before the osb activations (acts) so the
    # gather chain never head-of-line-blocks the Scalar queue.
    OUT_PARTS = [(0, 4), (4, 8), (8, 12), (12, 15), (15, 16)]

    with tile.TileContext(nc) as tc:
        with tc.tile_pool(name="wts", bufs=1) as wts, \
             tc.tile_pool(name="xdat", bufs=1) as xdat, \
             tc.tile_pool(name="xqs", bufs=2) as xqs, \
             tc.tile_pool(name="work", bufs=1) as work, \
             tc.tile_pool(name="opool", bufs=2) as opool, \
             tc.tile_pool(name="psX", bufs=3, space="PSUM") as psX, \
             tc.tile_pool(name="pkp", bufs=1, space="PSUM") as pkp:

            # ---- input DMAs: SEVEN host-packed mega-blobs ----
            # Each DMA on a ring costs ~1.5-2.3us of serialized latency
            # regardless of size, so the inputs are packed on the host into
            # one blob per (ring, priority-class):
            #   SP ring:  qpack (wq + xq chunk0), convB (conv kc2/3)
            #   ACT ring: convA (conv kc0/1) -- lands first, P2 m0 starts
            #   SWDGE:    wkpack (all conv weights), xqrest, cpb, ones
            qpack = wts.tile([128, 4096], BF16, tag="qpack")
            nc.sync.dma_start(out=qpack[:], in_=qpack_in[:])
            convA = xdat.tile([128, 8192], BF16, tag="convA")
            nc.scalar.dma_start(out=convA[:], in_=convA_in[:])
            convB = xdat.tile([128, 8192], BF16, tag="convB")
            nc.sync.dma_start(out=convB[:], in_=convB_in[:])
            wkpack = xdat.tile([128, 8192], BF16, tag="wkpack")
            nc.gpsimd.dma_start(out=wkpack[:], in_=wkpack_in[:])
            xqrest = xqs.tile([128, 6144], BF16, tag="xqrest")
            nc.gpsimd.dma_start(out=xqrest[:], in_=xqrest_in[:])
            cpb_t = wts.tile([128, 3 * MB], F32, tag="cpb")
            nc.gpsimd.dma_start(out=cpb_t[:], in_=cpb_in[:])
            ones = wts.tile([1, 128], F32R, tag="ones")
            nc.gpsimd.dma_start(out=ones[:], in_=ones_in[:])
            ck_t = cpb_t[:, 0:MB]
            pv_t = cpb_t[:, MB:2 * MB]
            bb_t = cpb_t[:, 2 * MB:3 * MB]

            # views into the packs
            wq_t = [qpack[:, kc * 512:(kc + 1) * 512] for kc in range(KC)]
            xq_t = {}
            for kc in range(KC):
                xq_t[(0, kc)] = qpack[:, 2048 + kc * 512:2048 + (kc + 1) * 512]
                for c in range(1, NCH):
                    xq_t[(c, kc)] = xqrest[:, kc * 1536:(kc + 1) * 1536]
            xce_t, xcd_t, wksr_t = {}, {}, {}
            for kc in range(KC):
                blob = convA if kc < 2 else convB
                base = (kc % 2) * 4096
                for e in range(4):
                    xce_t[(e, kc)] = blob[:, base + e * 512:
                                          base + (e + 1) * 512]
                    xcd_t[(e, kc)] = blob[:, base + 2048 + e * 512:
                                          base + 2048 + (e + 1) * 512]
                for m in range(MB):
                    wksr_t[(m, kc)] = wkpack[:, kc * 2048 + m * 512:
                                             kc * 2048 + (m + 1) * 512]

            # ---- persistent activations ----
            q_sb = [work.tile([128, T], BF16, tag=f"q{m}", name=f"q{m}")
                    for m in range(MB)]
            # k2 = [k_even | k_diff] per head pair
            k2_sb = [work.tile([128, NK], BF16, tag=f"k2{m}", name=f"k2{m}")
                     for m in range(MB)]
            s_acc = work.tile([128, TT * NH], F32, tag="sacc")
            s_cols = work.tile([128, TT], F32, tag="scols")
            sflat = work.tile([1, T], F32R, tag="sflat")

            # ---- P1 unit: q projection for one (chunk, head-pair) ----
            def emit_q_unit(c, m):
                pq = psX.tile([128, 1024], F32, tag="xbank",
                              name=f"pq{c}_{m}")
                for kc in range(KC):
                    xt = xq_t[(c, kc)]
                    rhs = (xt if c == 0
                           else xt[:, (c - 1) * 512:c * 512])
                    nc.tensor.matmul(
                        pq[:, 0:512],
                        wq_t[kc][:, m * 128:(m + 1) * 128],
                        rhs,
                        start=(kc == 0), stop=(kc == KC - 1))
                # split the PSUM->SBUF casts between the two egress engines
                dst = q_sb[m][:, c * 512:(c + 1) * 512]
                nc.vector.tensor_copy(dst, pq[:, 0:512])

            # ---- P2: k even/diff banks for head pair m (32 matmuls) ----
            def emit_k_mms(m, pk, units):
                for (e, kc) in units:
                    first = (e == 0 and kc == 0)
                    last = (e == 3 and kc == KC - 1)
                    first = (kc == 0 and e == 0)
                    last = (kc == KC - 1 and e == 3)
                    nc.tensor.matmul(
                        pk[:, 0:512],
                        wksr_t[(m, kc)][:, e * 128:(e + 1) * 128],
                        xce_t[(e, kc)],
                        start=first, stop=last)
                    nc.tensor.matmul(
                        pk[:, 512:1024],
                        wksr_t[(m, kc)][:, e * 128:(e + 1) * 128],
                        xcd_t[(e, kc)],
                        start=first, stop=last)

            def emit_k_act(m, pk):
                nc.scalar.activation(
                    k2_sb[m][:, 0:512], pk[:, 0:512], ACTF.Identity,
                    bias=ck_t[:, m:m + 1], scale=1.0)
                nc.scalar.copy(k2_sb[m][:, 512:1024], pk[:, 512:1024])

            # ---- P3: scores, software-pipelined over groups ----
            # front(g): diff-pair matmuls into pX (sets has_written), then
            #   relu pX IN PLACE (has_written survives the ACT overwrite).
            # finish(g): even-pair matmuls ACCUMULATE onto pX with
            #   start=False -> pX = relu(D)+E = pair max; then reduce_max.
            state = {}

            def emit_front(m, tt):
                tsl = slice(tt * 128, (tt + 1) * 128)
                qs = q_sb[m]
                pX = psX.tile([128, 1024], F32, tag="xbank",
                              name=f"pX{m}_{tt}")
                nc.tensor.matmul(pX[:, 0:512], qs[0:64, tsl],
                                 k2_sb[m][0:64, 512:1024], start=True,
                                 stop=True, tile_position=(0, 0))
                nc.tensor.matmul(pX[:, 512:1024], qs[64:128, tsl],
                                 k2_sb[m][64:128, 512:1024], start=True,
                                 stop=True, tile_position=(64, 0))
                if (m, tt) in VRELU:
                    nc.vector.tensor_scalar_max(pX[:], pX[:], 0.0)
                else:
                    nc.scalar.activation(pX[:], pX[:], ACTF.Relu)
                state[(m, tt)] = pX

            def emit_finish(m, tt):
                tsl = slice(tt * 128, (tt + 1) * 128)
                qs = q_sb[m]
                pX = state.pop((m, tt))
                nc.tensor.matmul(pX[:, 0:512], qs[0:64, tsl],
                                 k2_sb[m][0:64, 0:512], start=False,
                                 stop=True, tile_position=(0, 0))
                nc.tensor.matmul(pX[:, 512:1024], qs[64:128, tsl],
                                 k2_sb[m][64:128, 0:512], start=False,
                                 stop=True, tile_position=(64, 0))
                cols = slice(tt * NH + 2 * m, tt * NH + 2 * m + 2)
                nc.vector.reduce_max(
                    s_acc[:, cols],
                    pX[:].rearrange("p (a b) -> p a b", a=2), axis=AX.X)
                if m == MB - 1:
                    nc.vector.reduce_sum(
                        s_cols[:, tt:tt + 1],
                        s_acc[:, tt * NH:(tt + 1) * NH], axis=AX.X)

            # ---- outer: rank-1 output for a token range, two stages ----
            out_state = {}

            def outer_start(idx):
                lo_tt, hi_tt = OUT_PARTS[idx]
                ntok = (hi_tt - lo_tt) * 128
                with nc.named_scope("outer"):
                    sl = slice(lo_tt, hi_tt)
                    nc.sync.dma_start(out=sbounce[:, sl], in_=s_cols[:, sl])
                    nc.gpsimd.dma_start(
                        out=sflat[0:1, lo_tt * 128:hi_tt * 128],
                        in_=sbounce[:, sl].rearrange("p t -> () t p"))
                    # the pk pool is idle during the last pass; borrow it
                    pbc = pkp.tile([128, 1024], F32, tag="kbank",
                                   name=f"pbc{idx}")
                    for t2 in range((ntok + 511) // 512):
                        w = min(512, ntok - t2 * 512)
                        nc.tensor.matmul(
                            pbc[:, t2 * 512:t2 * 512 + w], ones[:],
                            sflat[0:1,
                                  lo_tt * 128 + t2 * 512:
                                  lo_tt * 128 + t2 * 512 + w],
                            start=True, stop=True)
                    out_state[idx] = pbc

            def outer_acts(idx):
                lo_tt, hi_tt = OUT_PARTS[idx]
                ntok = (hi_tt - lo_tt) * 128
                tok = slice(lo_tt * 128, hi_tt * 128)
                pbc = out_state.pop(idx)
                with nc.named_scope("outer"):
                    osb = opool.tile([128, MB * 512], F32, tag="osb",
                                     name=f"osb{idx}")
                    for m in range(MB):
                        nc.scalar.activation(
                            osb[:, m * ntok:(m + 1) * ntok],
                            pbc[:, 0:ntok], ACTF.Identity,
                            bias=bb_t[:, m:m + 1], scale=pv_t[:, m:m + 1])
                    deng = nc.sync if idx % 2 == 0 else nc.gpsimd
                    deng.dma_start(
                        out=out_ext[:, tok].rearrange("(m p) t -> p m t",
                                                      m=MB),
                        in_=osb[:, 0:MB * ntok].rearrange(
                            "p (m t) -> p m t", m=MB))

            # ------------------ emission schedule ------------------
            units_all = [(e, kc) for kc in range(KC) for e in range(4)]
            prev = [None]

            def do_group(m, tt):
                emit_front(m, tt)
                if prev[0] is not None:
                    emit_finish(*prev[0])
                prev[0] = (m, tt)

            # lead: P2 m0 rides the conv DMAs kc-by-kc, P1 c0 fills the
            # holes; the first half of P2 m1 also fits before pass 0.
            pk = pkp.tile([128, 1024], F32, tag="kbank", name="pk0")
            with nc.named_scope("lead"):
                for kc in range(KC):
                    emit_k_mms(0, pk, units_all[4 * kc:4 * kc + 4])
                    emit_q_unit(0, kc)
                emit_k_act(0, pk)
            pk = pkp.tile([128, 1024], F32, tag="kbank", name="pk1")
            emit_k_mms(1, pk, units_all[0:8])

            with nc.named_scope("scores0"):
                for tt in range(TT):
                    do_group(0, tt)
                    if tt < 4:
                        emit_k_mms(1, pk, units_all[8 + 2 * tt:10 + 2 * tt])
                    if tt == 4:
                        emit_k_act(1, pk)
                    if 3 <= tt <= 14:
                        u = tt - 3
                        emit_q_unit(1 + u // 4, u % 4)

            with nc.named_scope("scores1"):
                pk = pkp.tile([128, 1024], F32, tag="kbank", name="pk2")
                for tt in range(TT):
                    do_group(1, tt)
                    if tt < 8:
                        emit_k_mms(2, pk, units_all[2 * tt:2 * tt + 2])
                    if tt == 8:
                        emit_k_act(2, pk)
                        pk = pkp.tile([128, 1024], F32, tag="kbank",
                                      name="pk3")
                    if 8 <= tt:
                        emit_k_mms(3, pk, units_all[2 * (tt - 8):
                                                    2 * (tt - 8) + 2])
                emit_k_act(3, pk)

            # passes m2 + m3 interleaved by token tile: s_cols finalizes
            # progressively, so the output stage spreads over this whole
            # segment instead of piling up at the end.
            with nc.named_scope("scores23"):
                for tt in range(TT):
                    do_group(2, tt)
                    do_group(3, tt)
                    if tt == 4:
                        outer_start(0)
                    elif tt == 7:
                        outer_acts(0)
                    elif tt == 8:
                        outer_start(1)
                    elif tt == 11:
                        outer_acts(1)
                    elif tt == 12:
                        outer_start(2)
                    elif tt == 15:
                        outer_acts(2)
                        outer_start(3)
                emit_finish(*prev[0])
                outer_acts(3)
                outer_start(4)
                outer_acts(4)

    _split_multi_waits(nc)
    return nc


# ---------------------------------------------------------------------------
# host side
# ---------------------------------------------------------------------------
def _prep_host(x, Wq, Wk, Wsr, bsr, bn_gamma, bn_beta, bn_mean, bn_var,
               Wproj, bproj):
    import ml_dtypes
    bf16 = ml_dtypes.bfloat16
    f8 = np.float64
    scale = HD ** -0.5
    g = bn_gamma.astype(f8) / np.sqrt(bn_var.astype(f8) + EPS)
    A = Wk.astype(f8) * g[None, :]
    ck = A @ bsr.astype(f8) + Wk.astype(f8) @ (
        bn_beta.astype(f8) - bn_mean.astype(f8) * g)
    # wkpack [128, 8192]: [kc][m][e][128] columns
    wk4 = np.stack([
        (A @ Wsr[:, :, e // 2, e % 2].astype(f8)).T for e in range(4)
    ])                                                 # (4, C_in, C_out)
    wkpack = np.empty((128, 8192), np.float64)
    for kc in range(KC):
        for m in range(MB):
            for e in range(4):
                wkpack[:, kc * 2048 + m * 512 + e * 128:
                       kc * 2048 + m * 512 + (e + 1) * 128] = \
                    wk4[e][kc * 128:(kc + 1) * 128, m * 128:(m + 1) * 128]
    wkpack = wkpack.astype(bf16)
    wqT = (Wq.astype(f8) * scale).T.astype(bf16)       # (C_in, C_out)

    x4 = x.reshape(B, C, HW_, HW_)
    xce = np.empty((B, C, 4, NKE), np.float32)
    xcd = np.empty((B, C, 4, NKE), np.float32)
    for e in range(4):
        di, dj = e // 2, e % 2
        even = x4[:, :, di::2, dj::4].reshape(B, C, NKE)
        odd = x4[:, :, di::2, dj + 2::4].reshape(B, C, NKE)
        xce[:, :, e] = even
        xcd[:, :, e] = odd - even
    xce = xce.reshape(B, C, 4 * NKE)
    xcd = xcd.reshape(B, C, 4 * NKE)
    # convA/convB [128, 8192]: [kc-of-pair][even|odd diff][e][512]
    convA = np.empty((B, 128, 8192), np.float32)
    convB = np.empty((B, 128, 8192), np.float32)
    for kc in range(KC):
        blob = convA if kc < 2 else convB
        base = (kc % 2) * 4096
        rows = slice(kc * 128, (kc + 1) * 128)
        blob[:, :, base:base + 2048] = xce[:, rows]
        blob[:, :, base + 2048:base + 4096] = xcd[:, rows]
    convA = convA.astype(bf16)
    convB = convB.astype(bf16)

    v = x.astype(f8).mean(axis=2)                       # (B, C)
    pv = (Wproj.astype(f8) @ v.T).T.astype(np.float32)  # (B, C)

    ck_t = ck.astype(np.float32).reshape(MB, 128).T    # (128, MB)
    bb_t = bproj.astype(np.float32).reshape(MB, 128).T
    cpb = [np.concatenate(
        [ck_t, pv[b].reshape(MB, 128).T, bb_t], axis=1).astype(np.float32)
        for b in range(B)]                              # (128, 3*MB)
    # qpack [128, 4096] per (b, half): wq kc-major then xq chunk0
    # xqrest [128, 6144]: [kc][1536] tokens 512..2048
    wqT32 = wqT.astype(np.float32)
    qpack = np.empty((B, 2, 128, 4096), np.float32)
    xqrest = np.empty((B, 2, 128, 6144), np.float32)
    for half in range(2):
        xh = x[:, :, half * T:(half + 1) * T]
        for kc in range(KC):
            rows = slice(kc * 128, (kc + 1) * 128)
            qpack[:, half, :, kc * 512:(kc + 1) * 512] = wqT32[None, rows]
            qpack[:, half, :, 2048 + kc * 512:2048 + (kc + 1) * 512] = \
                xh[:, rows, 0:512]
            xqrest[:, half, :, kc * 1536:(kc + 1) * 1536] = \
                xh[:, rows, 512:T]
    qpack = qpack.astype(bf16)
    xqrest = xqrest.astype(bf16)
    return qpack, xqrest, convA, convB, wkpack, cpb


def kernel(x, y, Wq, Wk, Wsr, bsr, bn_gamma, bn_beta, bn_mean, bn_var,
           Wproj, bproj, H, W):
    x = np.asarray(x, np.float32)
    qpack, xqrest, convA, convB, wkpack, cpb = _prep_host(
        x, np.asarray(Wq, np.float32), np.asarray(Wk, np.float32),
        np.asarray(Wsr, np.float32), np.asarray(bsr, np.float32),
        np.asarray(bn_gamma, np.float32), np.asarray(bn_beta, np.float32),
        np.asarray(bn_mean, np.float32), np.asarray(bn_var, np.float32),
        np.asarray(Wproj, np.float32), np.asarray(bproj, np.float32))

    _install_fixes()
    _install_ntff_hook()
    from concourse.bass_utils import run_bass_kernel_spmd

    if "nc" not in _cache:
        _cache["nc"] = _build_program()
    nc = _cache["nc"]

    ones = np.ones((1, 128), np.float32)
    in_maps = []
    for core in range(NCORES):
        b, half = core // 2, core % 2
        in_maps.append({
            "qpack": qpack[b, half], "xqrest": xqrest[b, half],
            "convA": convA[b], "convB": convB[b], "wkpack": wkpack,
            "cpb": cpb[b],
            "ones": ones,
        })

    trace = os.environ.get("BASS_KERNEL_TRACE", "0") == "1"
    res = run_bass_kernel_spmd(nc, in_maps, list(range(NCORES)), trace=trace)
    if trace:
        print(f"HW exec time: {res.exec_time_ns} ns")
        _cache["last_exec_time_ns"] = res.exec_time_ns
        _cache["last_trace"] = res.instructions_and_trace

    out = np.empty((B, C, N), np.float32)
    for core in range(NCORES):
        b, half = core // 2, core % 2
        out[b][:, half * T:(half + 1) * T] = res.results[core]["out"]
    return out



# revision 7
# speedup vs baseline: 1.0457x; 1.0457x over previous
"""Trainium2 Bass kernel for nn_Attention_Joint_MaxPool.

Math (see reference):
  q = (Wq*scale) @ x                        (B, C, N), heads on rows
  xsr = conv2x2s2(x) ; k = Wk @ BN(xsr)     (B, C, Nk=1024)
  attn = max over keys of q_h . k_h         (B, NH, N)
  s = sum over heads of attn                (B, N)
  out[b,c,n] = (Wproj @ mean_n x)[c] * s[b,n] + bproj[c]

Weight folding done on host:
  g = gamma/sqrt(var+eps); A = Wk * g[None,:]
  k = sum_e (A @ Wsr[:,:,e]) @ x_sub[e] + ck,  ck = A@bsr + Wk@(beta-mean*g)
  pv[b] = Wproj @ mean_n x[b]   (rank-1 output structure)

Key structure (v2):
  All heavy matmuls run in bf16.  Max over keys via the pair cascade
  max(a,b) = a + relu(b-a): diff-pair matmuls write a PSUM bank, ScalarE
  relus it IN PLACE (has_written survives), even-pair matmuls accumulate
  onto it with start=False, VectorE reduce_maxes the bank.  The two heads
  of a pair run CONCURRENTLY in the PE via tile_position row pairing
  (measured: second matmul of a pair retires ~4ns after the first).

  Scheduling (the v2 changes):
  - all input DMAs ride ONE HWDGE ring in strict priority order
    (cpb -> wq+xq0+wk01 -> convA -> xq123 -> convB -> wk23) so the
    k-path data lands ~21us in instead of ~28us (round-robin starved).
  - PE warmup burst on memset junk so HAM is at 8/8 before real work.
  - k m1 accumulates in a borrowed score-pool PSUM tile during the lead.
  - passes: A (m=0), B (m=1), then CD (m=2,3 interleaved per tile), all
    with a 2-deep front/finish software pipeline so the ScalarE relu
    latency never head-of-line-blocks the PE.
  - k m2/m3 matmuls and the q-projection units are spread across pass
    A/B slots with explicit data deadlines.
  - output stage: 8 parts of 2 token tiles each, emitted as soon as the
    pair of tiles finalizes in pass CD; output DMAs on the scalar ring.

Sharding: 8 cores; core i -> batch i//2, token half i%2 (2048 tokens).
Each core is fully independent (no collectives).
"""

import os
import sys
import types
import numpy as np

# ---------------------------------------------------------------------------
# problem constants (hardcoded; kernel.py must be self-contained)
# ---------------------------------------------------------------------------
B, C, N = 4, 512, 4096
NH, HD = 8, 64
SR = 2
EPS = 1e-5
HW_ = 64                      # H = W = 64
T = N // 2                    # tokens per core
NK = 1024                     # conv output positions (keys)
NKE = NK // 2                 # even keys
MB = C // 128                 # 4 channel blocks
KC = C // 128                 # 4 contraction chunks
NCORES = 8
TT = T // 128                 # 16 token tiles per core
NCH = T // 512                # 4 q chunks per core

_cache = {}


# ---------------------------------------------------------------------------
# workarounds for this container's toolchain
# ---------------------------------------------------------------------------
def _install_fixes():
    import concourse.tile as tile
    import concourse.mybir as mybir
    from concourse.vector_clock import ScopedClock

    if getattr(tile.TileContext, "_drain_patched", False):
        return

    def _patched_drain_and_barrier(self, tick_clock, wait_clock):
        nc = self.nc
        probe = nc.sync.nop(nofuse=True, hint="drain_wait_carrier")
        wait_clock.add_sem_waits(
            probe.ins, ScopedClock({None: tick_clock.global_clock})
        )
        waits = list(probe.ins.sync_info.on_wait) if probe.ins.sync_info else []
        if len(waits) > 1:
            probe.ins.sync_info = mybir.SyncInfo(on_wait=waits[:1], on_update=[])
            for w in waits[1:]:
                extra = nc.sync.nop(nofuse=True, hint="drain_wait_carrier")
                extra.ins.sync_info = mybir.SyncInfo(on_wait=[w], on_update=[])
        nc.sync.drain()
        nc.all_engine_barrier()
        assert self.sems is not None
        popped = nc._tile_sem_poison_stack.pop()
        assert popped is self._sem_poison
        nc.clear_and_free_semaphores(list(self.sems.allocated().values()))
        nc.all_engine_barrier()

    tile.TileContext._drain_and_barrier = _patched_drain_and_barrier
    tile.TileContext._drain_patched = True


def _split_multi_waits(nc):
    """This walrus build allows only one sync-wait per instruction; hoist
    extra waits onto same-engine nops inserted just before the instruction."""
    import concourse.mybir as mybir

    ctr = 0
    for f in nc.m.functions:
        for bb in f.blocks:
            changed = False
            out = []
            for inst in bb.instructions:
                si = inst.sync_info
                tname = type(inst).__name__
                if (si is not None and si.on_wait and len(si.on_wait) > 1
                        and "Collective" not in tname):
                    waits = list(si.on_wait)
                    for w in waits[:-1]:
                        ctr += 1
                        nop = mybir.InstNoOp(
                            name=f"I-ws-{ctr}",
                            engine=inst.engine,
                            sync_info=mybir.SyncInfo(on_wait=[w], on_update=[]),
                        )
                        nc.register_instruction(nop, overwrite=True)
                        out.append(nop)
                    inst.sync_info = mybir.SyncInfo(
                        on_wait=waits[-1:], on_update=list(si.on_update)
                    )
                    changed = True
                out.append(inst)
            if changed:
                bb.instructions = out


def _install_ntff_hook():
    """Provide antenv.axon_hooks (missing in this image) so trace=True works."""
    try:
        from antenv import axon_hooks  # noqa: F401
        return
    except ImportError:
        pass
    try:
        import antenv
        from trn_agent_boot.trn_boot import _ntff_profile_via_ctypes
    except ImportError:
        return
    mod = types.ModuleType("antenv.axon_hooks")
    _hook = [None]
    mod.set_axon_ntff_profile_hook = lambda h: _hook.__setitem__(0, h)
    mod.get_axon_ntff_profile_hook = lambda: _hook[0]
    sys.modules["antenv.axon_hooks"] = mod
    antenv.axon_hooks = mod
    mod.set_axon_ntff_profile_hook(
        _ntff_profile_via_ctypes("/opt/axon/libaxon_pjrt.so")
    )


# ---------------------------------------------------------------------------
# device program
# ---------------------------------------------------------------------------
def _build_program():
    import concourse.bass as bass
    import concourse.mybir as mybir
    import concourse.tile as tile

    F32 = mybir.dt.float32
    F32R = mybir.dt.float32r
    BF16 = mybir.dt.bfloat16
    AX = mybir.AxisListType
    ACTF = mybir.ActivationFunctionType
    ALU = mybir.AluOpType

    nc = bass.Bass()

    cpb_in = nc.declare_dram_parameter("cpb", [128, 3 * MB], F32, isOutput=False)
    pack1_in = nc.declare_dram_parameter("pack1", [128, 8192], BF16,
                                         isOutput=False)
    convA_in = nc.declare_dram_parameter("convA", [128, 8192], BF16,
                                         isOutput=False)
    xq123_in = nc.declare_dram_parameter("xq123", [128, 6144], BF16,
                                         isOutput=False)
    convB_in = nc.declare_dram_parameter("convB", [128, 8192], BF16,
                                         isOutput=False)
    wk23_in = nc.declare_dram_parameter("wk23", [128, 4096], BF16,
                                        isOutput=False)
    ones_in = nc.declare_dram_parameter("ones", [1, 128], F32R, isOutput=False)
    out_ext = nc.declare_dram_parameter("out", [C, T], F32, isOutput=True)

    sbounce = nc.dram_tensor("sbounce", [128, TT], F32)

    NPARTS = 8
    PTILES = 2                 # token tiles per output part

    with tile.TileContext(nc) as tc:
        with tc.tile_pool(name="wts", bufs=1) as wts, \
             tc.tile_pool(name="xdat", bufs=1) as xdat, \
             tc.tile_pool(name="work", bufs=1) as work, \
             tc.tile_pool(name="opool", bufs=2) as opool, \
             tc.tile_pool(name="psX", bufs=3, space="PSUM") as psX, \
             tc.tile_pool(name="pkp", bufs=1, space="PSUM") as pkp:

            # ---- input DMAs: strict priority chain on the SP HWDGE ring ----
            cpb_t = wts.tile([128, 3 * MB], F32, tag="cpb")
            nc.sync.dma_start(out=cpb_t[:], in_=cpb_in[:])
            pack1 = wts.tile([128, 8192], BF16, tag="pack1")
            nc.sync.dma_start(out=pack1[:], in_=pack1_in[:])
            convA = xdat.tile([128, 8192], BF16, tag="convA")
            nc.sync.dma_start(out=convA[:], in_=convA_in[:])
            xq123 = wts.tile([128, 6144], BF16, tag="xq123")
            nc.sync.dma_start(out=xq123[:], in_=xq123_in[:])
            convB = xdat.tile([128, 8192], BF16, tag="convB")
            nc.sync.dma_start(out=convB[:], in_=convB_in[:])
            wk23 = xdat.tile([128, 4096], BF16, tag="wk23")
            nc.sync.dma_start(out=wk23[:], in_=wk23_in[:])
            ones = wts.tile([1, 128], F32R, tag="ones")
            nc.sync.dma_start(out=ones[:], in_=ones_in[:])

            ck_t = cpb_t[:, 0:MB]
            pv_t = cpb_t[:, MB:2 * MB]
            bb_t = cpb_t[:, 2 * MB:3 * MB]

            # warmup junk (memset; no DMA dependency)
            warm = wts.tile([128, 128], BF16, tag="warm")
            nc.gpsimd.memset(warm[:], 0.0)

            # views
            wq_t = [pack1[:, kc * 512:(kc + 1) * 512] for kc in range(KC)]
            xq_t = {}
            for kc in range(KC):
                xq_t[(0, kc)] = pack1[:, 2048 + kc * 512:2048 + (kc + 1) * 512]
                for c in range(1, NCH):
                    xq_t[(c, kc)] = xq123[:, (c - 1) * 2048 + kc * 512:
                                          (c - 1) * 2048 + (kc + 1) * 512]
            xce_t, xcd_t, wksr_t = {}, {}, {}
            for kc in range(KC):
                blob = convA if kc < 2 else convB
                base = (kc % 2) * 4096
                for e in range(4):
                    xce_t[(e, kc)] = blob[:, base + e * 512:
                                          base + (e + 1) * 512]
                    xcd_t[(e, kc)] = blob[:, base + 2048 + e * 512:
                                          base + 2048 + (e + 1) * 512]
                for m in range(MB):
                    if m < 2:
                        wksr_t[(m, kc)] = pack1[:, 4096 + m * 2048 + kc * 512:
                                                4096 + m * 2048 + (kc + 1) * 512]
                    else:
                        wksr_t[(m, kc)] = wk23[:, (m - 2) * 2048 + kc * 512:
                                               (m - 2) * 2048 + (kc + 1) * 512]

            # ---- persistent activations ----
            q_sb = [work.tile([128, T], BF16, tag=f"q{m}", name=f"q{m}")
                    for m in range(MB)]
            k2_sb = [work.tile([128, NK], BF16, tag=f"k2{m}", name=f"k2{m}")
                     for m in range(MB)]
            s_acc = work.tile([128, TT * NH], F32, tag="sacc")
            s_cols = work.tile([128, TT], F32, tag="scols")
            sflat = work.tile([1, T], F32R, tag="sflat")

            qcast_ctr = [0]

            # ---- q projection unit for one (chunk, head-pair) ----
            def emit_q_unit(c, m):
                pq = psX.tile([128, 1024], F32, tag="xbank",
                              name=f"pq{c}_{m}")
                for kc in range(KC):
                    nc.tensor.matmul(
                        pq[:, 0:512],
                        wq_t[kc][:, m * 128:(m + 1) * 128],
                        xq_t[(c, kc)],
                        start=(kc == 0), stop=(kc == KC - 1))
                dst = q_sb[m][:, c * 512:(c + 1) * 512]
                # split the PSUM->SBUF casts between the two egress engines
                if qcast_ctr[0] % 4 == 3:
                    nc.scalar.copy(dst, pq[:, 0:512])
                else:
                    nc.vector.tensor_copy(dst, pq[:, 0:512])
                qcast_ctr[0] += 1

            # ---- k even/diff banks for head pair m ----
            def emit_k_mms(m, pk, units):
                for (e, kc) in units:
                    first = (kc == 0 and e == 0)
                    last = (kc == KC - 1 and e == 3)
                    nc.tensor.matmul(
                        pk[:, 0:512],
                        wksr_t[(m, kc)][:, e * 128:(e + 1) * 128],
                        xce_t[(e, kc)],
                        start=first, stop=last)
                    nc.tensor.matmul(
                        pk[:, 512:1024],
                        wksr_t[(m, kc)][:, e * 128:(e + 1) * 128],
                        xcd_t[(e, kc)],
                        start=first, stop=last)

            def emit_k_act(m, pk):
                nc.scalar.activation(
                    k2_sb[m][:, 0:512], pk[:, 0:512], ACTF.Identity,
                    bias=ck_t[:, m:m + 1], scale=1.0)
                nc.scalar.copy(k2_sb[m][:, 512:1024], pk[:, 512:1024])

            # ---- score groups: front (diff+relu) / finish (even+reduce) ----
            state = {}

            def emit_front(m, tt):
                tsl = slice(tt * 128, (tt + 1) * 128)
                qs = q_sb[m]
                pX = psX.tile([128, 1024], F32, tag="xbank",
                              name=f"pX{m}_{tt}")
                nc.tensor.matmul(pX[:, 0:512], qs[0:64, tsl],
                                 k2_sb[m][0:64, 512:1024], start=True,
                                 stop=True, tile_position=(0, 0))
                nc.tensor.matmul(pX[:, 512:1024], qs[64:128, tsl],
                                 k2_sb[m][64:128, 512:1024], start=True,
                                 stop=True, tile_position=(64, 0))
                nc.scalar.activation(pX[:], pX[:], ACTF.Relu)
                state[(m, tt)] = pX

            def emit_finish(m, tt):
                tsl = slice(tt * 128, (tt + 1) * 128)
                qs = q_sb[m]
                pX = state.pop((m, tt))
                nc.tensor.matmul(pX[:, 0:512], qs[0:64, tsl],
                                 k2_sb[m][0:64, 0:512], start=False,
                                 stop=True, tile_position=(0, 0))
                nc.tensor.matmul(pX[:, 512:1024], qs[64:128, tsl],
                                 k2_sb[m][64:128, 0:512], start=False,
                                 stop=True, tile_position=(64, 0))
                cols = slice(tt * NH + 2 * m, tt * NH + 2 * m + 2)
                nc.vector.reduce_max(
                    s_acc[:, cols],
                    pX[:].rearrange("p (a b) -> p a b", a=2), axis=AX.X)
                if m == MB - 1:
                    nc.vector.reduce_sum(
                        s_cols[:, tt:tt + 1],
                        s_acc[:, tt * NH:(tt + 1) * NH], axis=AX.X)

            # ---- rank-1 output stage: 8 parts of 2 token tiles ----
            out_state = {}

            def outer_start(p):
                lo_tt, hi_tt = p * PTILES, (p + 1) * PTILES
                ntok = PTILES * 128
                with nc.named_scope("outer"):
                    sl = slice(lo_tt, hi_tt)
                    nc.sync.dma_start(out=sbounce[:, sl], in_=s_cols[:, sl])
                    nc.gpsimd.dma_start(
                        out=sflat[0:1, lo_tt * 128:hi_tt * 128],
                        in_=sbounce[:, sl].rearrange("p t -> () t p"))
                    pbc = pkp.tile([128, 1024], F32, tag="kbank",
                                   name=f"pbc{p}")
                    nc.tensor.matmul(
                        pbc[:, 0:ntok], ones[:],
                        sflat[0:1, lo_tt * 128:hi_tt * 128],
                        start=True, stop=True)
                    out_state[p] = pbc

            def outer_acts(p):
                lo_tt, hi_tt = p * PTILES, (p + 1) * PTILES
                ntok = PTILES * 128
                tok = slice(lo_tt * 128, hi_tt * 128)
                pbc = out_state.pop(p)
                with nc.named_scope("outer"):
                    osb = opool.tile([128, MB * ntok], F32, tag="osb",
                                     name=f"osb{p}")
                    for m in range(MB):
                        if m % 2 == 0:
                            nc.scalar.activation(
                                osb[:, m * ntok:(m + 1) * ntok],
                                pbc[:, 0:ntok], ACTF.Identity,
                                bias=bb_t[:, m:m + 1], scale=pv_t[:, m:m + 1])
                        else:
                            nc.vector.tensor_scalar(
                                osb[:, m * ntok:(m + 1) * ntok],
                                pbc[:, 0:ntok],
                                pv_t[:, m:m + 1], bb_t[:, m:m + 1],
                                op0=ALU.mult, op1=ALU.add)
                    nc.scalar.dma_start(
                        out=out_ext[:, tok].rearrange("(m p) t -> p m t",
                                                      m=MB),
                        in_=osb[:, 0:MB * ntok].rearrange(
                            "p (m t) -> p m t", m=MB))

            # ------------------ emission schedule ------------------
            units_of = lambda kcs: [(e, kc) for kc in kcs for e in range(4)]

            # warmup: ~36 junk MMs to trip HAM to 8/8 before real work
            pkw = pkp.tile([128, 1024], F32, tag="kbank", name="pkwarm")
            with nc.named_scope("warm"):
                for i in range(36):
                    nc.tensor.matmul(pkw[:, 0:128], warm[:], warm[:],
                                     start=True, stop=True)

            # lead
            with nc.named_scope("lead"):
                for m in range(MB):
                    emit_q_unit(0, m)          # pack1 (q chunk 0, all pairs)
                pk0 = pkp.tile([128, 1024], F32, tag="kbank", name="pk0")
                xbm1 = psX.tile([128, 1024], F32, tag="xbank", name="xbm1")
                # convA: kc0/1 for m0 and m1, interleaved
                for kc in (0, 1):
                    emit_k_mms(0, pk0, units_of([kc]))
                    emit_k_mms(1, xbm1, units_of([kc]))
                emit_q_unit(1, 0)              # xq123
                emit_q_unit(1, 1)
                for kc in (2, 3):              # convB
                    emit_k_mms(0, pk0, units_of([kc]))
                    emit_k_mms(1, xbm1, units_of([kc]))
                emit_k_act(0, pk0)
                emit_k_act(1, xbm1)

            # score passes with 2-deep pipeline
            pending = []

            def do_group(m, tt):
                emit_front(m, tt)
                pending.append((m, tt))
                if len(pending) > 2:
                    emit_finish(*pending.pop(0))

            def drain():
                while pending:
                    emit_finish(*pending.pop(0))

            # pass A: m=0; fillers: k m2 (kc0..3), q units
            pkA = pkp.tile([128, 1024], F32, tag="kbank", name="pk2")
            fillA = {
                0: lambda: emit_q_unit(1, 2),
                1: lambda: emit_k_mms(2, pkA, units_of([0])[0:2]),
                2: lambda: emit_k_mms(2, pkA, units_of([0])[2:4]),
                3: lambda: emit_k_mms(2, pkA, units_of([1])[0:2]),
                4: lambda: emit_k_mms(2, pkA, units_of([1])[2:4]),
                5: lambda: emit_q_unit(2, 0),
                6: lambda: emit_k_mms(2, pkA, units_of([2])[0:2]),
                7: lambda: emit_k_mms(2, pkA, units_of([2])[2:4]),
                8: lambda: emit_q_unit(3, 0),
                9: lambda: emit_k_mms(2, pkA, units_of([3])[0:2]),
                10: lambda: (emit_k_mms(2, pkA, units_of([3])[2:4]),
                             emit_k_act(2, pkA)),
                11: lambda: emit_q_unit(2, 1),
            }
            with nc.named_scope("passA"):
                for tt in range(TT):
                    do_group(0, tt)
                    f = fillA.get(tt)
                    if f:
                        f()

            # pass B: m=1; fillers: k m3, remaining early q units
            pkB = pkp.tile([128, 1024], F32, tag="kbank", name="pk3")
            fillB = {
                0: lambda: emit_k_mms(3, pkB, units_of([0])[0:2]),
                1: lambda: emit_k_mms(3, pkB, units_of([0])[2:4]),
                2: lambda: emit_q_unit(1, 3),
                3: lambda: emit_k_mms(3, pkB, units_of([1])[0:2]),
                4: lambda: emit_k_mms(3, pkB, units_of([1])[2:4]),
                5: lambda: emit_q_unit(2, 2),
                6: lambda: emit_k_mms(3, pkB, units_of([2])[0:2]),
                7: lambda: emit_k_mms(3, pkB, units_of([2])[2:4]),
                8: lambda: emit_q_unit(3, 1),
                9: lambda: emit_k_mms(3, pkB, units_of([3])[0:2]),
                10: lambda: (emit_k_mms(3, pkB, units_of([3])[2:4]),
                             emit_k_act(3, pkB)),
                11: lambda: emit_q_unit(2, 3),
            }
            with nc.named_scope("passB"):
                for tt in range(TT):
                    do_group(1, tt)
                    f = fillB.get(tt)
                    if f:
                        f()

            # pass CD: m=2,3 interleaved per tile; output parts ride along.
            # emit_finish(3, tt) lands 2 groups after emit_front(3, tt); the
            # part for tiles (2p, 2p+1) is emitted once finish(3, 2p+1) has
            # been emitted.
            fin_count = [0]
            fin_hooks = {}

            def do_group_cd(m, tt):
                emit_front(m, tt)
                pending.append((m, tt))
                if len(pending) > 2:
                    g = pending.pop(0)
                    emit_finish(*g)
                    fin_count[0] += 1
                    h = fin_hooks.pop(g, None)
                    if h:
                        h()

            with nc.named_scope("passCD"):
                for p in range(NPARTS):
                    fin_hooks[(3, p * PTILES + PTILES - 1)] = \
                        (lambda pp: lambda: (outer_start(pp)))(p)
                for tt in range(TT):
                    do_group_cd(2, tt)
                    if tt == 2:
                        emit_q_unit(3, 2)
                    do_group_cd(3, tt)
                    if tt == 6:
                        emit_q_unit(3, 3)
                    # drain output acts one part behind the start
                    if tt >= 3 and tt % 2 == 1:
                        p = (tt - 3) // 2
                        if p in out_state:
                            outer_acts(p)
                drain()
                for g, h in list(fin_hooks.items()):
                    h()
                for p in range(NPARTS):
                    if p in out_state:
                        outer_acts(p)

    _split_multi_waits(nc)
    return nc


# ---------------------------------------------------------------------------
# host side
# ---------------------------------------------------------------------------
def _prep_host(x, Wq, Wk, Wsr, bsr, bn_gamma, bn_beta, bn_mean, bn_var,
               Wproj, bproj):
    import ml_dtypes
    bf16 = ml_dtypes.bfloat16
    f8 = np.float64
    scale = HD ** -0.5
    g = bn_gamma.astype(f8) / np.sqrt(bn_var.astype(f8) + EPS)
    A = Wk.astype(f8) * g[None, :]
    ck = A @ bsr.astype(f8) + Wk.astype(f8) @ (
        bn_beta.astype(f8) - bn_mean.astype(f8) * g)
    # k weights: wk4[e] = (A @ Wsr[:,:,e]).T   (C_in, C_out)
    wk4 = np.stack([
        (A @ Wsr[:, :, e // 2, e % 2].astype(f8)).T for e in range(4)
    ])
    # wkblk[m][kc][e] = wk4[e][kc*128:(kc+1)*128, m*128:(m+1)*128]
    def wkblk(m):
        cols = np.empty((128, 2048), np.float64)
        for kc in range(KC):
            for e in range(4):
                cols[:, kc * 512 + e * 128:kc * 512 + (e + 1) * 128] = \
                    wk4[e][kc * 128:(kc + 1) * 128, m * 128:(m + 1) * 128]
        return cols

    wqT = (Wq.astype(f8) * scale).T                    # (C_in, C_out)

    x4 = x.reshape(B, C, HW_, HW_)
    xce = np.empty((B, C, 4, NKE), np.float32)
    xcd = np.empty((B, C, 4, NKE), np.float32)
    for e in range(4):
        di, dj = e // 2, e % 2
        even = x4[:, :, di::2, dj::4].reshape(B, C, NKE)
        odd = x4[:, :, di::2, dj + 2::4].reshape(B, C, NKE)
        xce[:, :, e] = even
        xcd[:, :, e] = odd - even
    xce = xce.reshape(B, C, 4 * NKE)
    xcd = xcd.reshape(B, C, 4 * NKE)
    # convA/convB [128, 8192]: [kc-of-pair][even|diff][e][512]
    convA = np.empty((B, 128, 8192), np.float32)
    convB = np.empty((B, 128, 8192), np.float32)
    for kc in range(KC):
        blob = convA if kc < 2 else convB
        base = (kc % 2) * 4096
        rows = slice(kc * 128, (kc + 1) * 128)
        blob[:, :, base:base + 2048] = xce[:, rows]
        blob[:, :, base + 2048:base + 4096] = xcd[:, rows]
    convA = convA.astype(bf16)
    convB = convB.astype(bf16)

    v = x.astype(f8).mean(axis=2)                       # (B, C)
    pv = (Wproj.astype(f8) @ v.T).T.astype(np.float32)  # (B, C)

    ck_t = ck.astype(np.float32).reshape(MB, 128).T    # (128, MB)
    bb_t = bproj.astype(np.float32).reshape(MB, 128).T
    cpb = [np.concatenate(
        [ck_t, pv[b].reshape(MB, 128).T, bb_t], axis=1).astype(np.float32)
        for b in range(B)]                              # (128, 3*MB)

    # pack1 [128, 8192]: wq kc-major (2048) | xq chunk0 (2048) | wk m0,m1
    # xq123 [128, 6144]: [c-1][kc][512]
    # wk23  [128, 4096]: [m-2][kc][e][128]
    pack1 = np.empty((B, 2, 128, 8192), np.float32)
    xq123 = np.empty((B, 2, 128, 6144), np.float32)
    for half in range(2):
        xh = x[:, :, half * T:(half + 1) * T]
        for kc in range(KC):
            rows = slice(kc * 128, (kc + 1) * 128)
            pack1[:, half, :, kc * 512:(kc + 1) * 512] = wqT[None, rows]
            pack1[:, half, :, 2048 + kc * 512:2048 + (kc + 1) * 512] = \
                xh[:, rows, 0:512]
            for c in range(1, NCH):
                xq123[:, half, :, (c - 1) * 2048 + kc * 512:
                      (c - 1) * 2048 + (kc + 1) * 512] = \
                    xh[:, rows, c * 512:(c + 1) * 512]
    pack1[:, :, :, 4096:6144] = wkblk(0)[None, None]
    pack1[:, :, :, 6144:8192] = wkblk(1)[None, None]
    wk23 = np.concatenate([wkblk(2), wkblk(3)], axis=1).astype(bf16)
    pack1 = pack1.astype(bf16)
    xq123 = xq123.astype(bf16)
    return cpb, pack1, convA, xq123, convB, wk23


def kernel(x, y, Wq, Wk, Wsr, bsr, bn_gamma, bn_beta, bn_mean, bn_var,
           Wproj, bproj, H, W):
    x = np.asarray(x, np.float32)
    cpb, pack1, convA, xq123, convB, wk23 = _prep_host(
        x, np.asarray(Wq, np.float32), np.asarray(Wk, np.float32),
        np.asarray(Wsr, np.float32), np.asarray(bsr, np.float32),
        np.asarray(bn_gamma, np.float32), np.asarray(bn_beta, np.float32),
        np.asarray(bn_mean, np.float32), np.asarray(bn_var, np.float32),
        np.asarray(Wproj, np.float32), np.asarray(bproj, np.float32))

    _install_fixes()
    _install_ntff_hook()
    from concourse.bass_utils import run_bass_kernel_spmd

    if "nc" not in _cache:
        _cache["nc"] = _build_program()
    nc = _cache["nc"]

    in_maps = []
    for core in range(NCORES):
        b, half = core // 2, core % 2
        in_maps.append({
            "cpb": cpb[b],
            "pack1": pack1[b, half],
            "convA": convA[b],
            "xq123": xq123[b, half],
            "convB": convB[b],
            "wk23": wk23,
            "ones": np.ones((1, 128), np.float32),
        })

    trace = os.environ.get("BASS_KERNEL_TRACE", "0") == "1"
    res = run_bass_kernel_spmd(nc, in_maps, list(range(NCORES)), trace=trace)
    if trace:
        print(f"HW exec time: {res.exec_time_ns} ns")
        _cache["last_exec_time_ns"] = res.exec_time_ns
        _cache["last_trace"] = res.instructions_and_trace

    out = np.empty((B, C, N), np.float32)
    for core in range(NCORES):
        b, half = core // 2, core % 2
        out[b][:, half * T:(half + 1) * T] = res.results[core]["out"]
    return out
